# revision 4
# baseline (speedup 1.0000x reference)
"""2-layer GCN on 8 Trainium2 NeuronCores (Bass/Tile, SPMD).

softmax(A @ relu(A @ (X@W1) + b1) @ W2 + b2), N=50k nodes, E=800k edges.

Strategy (1D graph partition, fp16 pair-table gathers on 4 SWDGE queues):
- Nodes get a global degree rank; rank k -> core k%8, local pos k//8, so all
  cores hold near-identical degree profiles and within-core order is
  degree-descending (tight per-tile max in-degree).
- Edges partitioned by dst owner. Slot grid per core: tile t (128 dsts),
  chunk j = j-th in-edge of each dst; per-tile chunk count D_t is the
  cross-core max in-degree of that tile (degree sort keeps padding ~7%).
- Gather tables are fp16 with rows PAIRED: element = 256B = [row of tile 2u
  | row of tile 2u+1] at partition q; pair index o*3200 + q*25 + u < 25600
  fits int16 (no low/high split). The weighted select between the two
  halves uses two per-slot weight grids (we: half0, wo: half1).
- dma_gather calls (<=8 chunks = 1024 idxs) round-robin over 4 SWDGE
  queues; Q7 descriptor generation for different queues runs on different
  GpSimd core pairs concurrently (~3x issue throughput vs 1 queue).
- Layer-2 table HW2 is 16 classes padded to the same 64-col pair layout, so
  both layers share identical idx/weight grids.
- XW1/HW2 shards are fp16 [128, 3200] built in SBUF, one DMA out, AllGather
  (half the bytes of fp32), gathered as [25600, 128] pair-rows.
"""

import os
import sys

sys.path.insert(0, "/opt/trn_rl_repo")

import numpy as np

N = 50000
E = 800000
F = 512
HID = 64
NCLS = 16
NCORES = 8
P = 128
NPC = N // NCORES  # 6250
TILES = 49
TROWS = TILES * P  # 6272
UPAIRS = 25  # ceil(TILES/2) pair-ranks per core
PAIRS_PC = UPAIRS * P  # 3200
NPAIRSG = NCORES * PAIRS_PC  # 25600
STAGE_CAP = 48  # chunks per stage (6 gather calls)
CPC = 8  # chunks per gather call (1024 idxs)

_TRACE = False
LAST_EXEC_NS = None


def _preprocess(src, dst, edge_weight):
    src = np.asarray(src).astype(np.int64).ravel()
    dst = np.asarray(dst).astype(np.int64).ravel()
    w = np.asarray(edge_weight).astype(np.float32).ravel()

    tdeg = np.bincount(dst, minlength=N)
    grank = np.empty(N, dtype=np.int64)
    grank[np.argsort(-tdeg, kind="stable")] = np.arange(N)
    owner = grank % NCORES
    pos = grank // NCORES  # degree-descending within core

    # node -> gather pair index / half
    tl = pos >> 7
    q = pos & 127
    u = tl >> 1
    pair_idx = owner * PAIRS_PC + q * UPAIRS + u  # < 25600
    half = tl & 1

    owner_dst = owner[dst]

    # per-tile chunk counts: cross-core max of per-node in-degree maxima
    D_t = np.zeros(TILES, dtype=np.int64)
    per_core = []
    for r in range(NCORES):
        m = owner_dst == r
        es, ew = src[m], w[m]
        dl = pos[dst[m]]
        cnt = np.bincount(dl, minlength=NPC)
        cnt_pad = np.concatenate([cnt, np.zeros(TROWS - NPC, np.int64)])
        D_t = np.maximum(D_t, cnt_pad.reshape(TILES, P).max(1))
        per_core.append((es, ew, dl))

    C0 = np.zeros(TILES, dtype=np.int64)  # tile -> first chunk column
    C0[1:] = np.cumsum(D_t)[:-1]
    ctot = int(D_t.sum())

    # stages: consecutive tiles, <= STAGE_CAP chunks each
    stages = []  # (t0, t1, c0, c1)
    t0 = 0
    while t0 < TILES:
        t1 = t0
        nch = 0
        while t1 < TILES and nch + D_t[t1] <= STAGE_CAP:
            nch += D_t[t1]
            t1 += 1
        if t1 == t0:  # single tile bigger than cap (shouldn't happen)
            t1 = t0 + 1
            nch = D_t[t0]
        stages.append((t0, t1, int(C0[t0]), int(C0[t0] + nch)))
        t0 = t1

    idx_grids, we_grids, wo_grids = [], [], []
    for r in range(NCORES):
        es, ew, dl = per_core[r]
        order = np.argsort(dl, kind="stable")
        sd, sw_, se = dl[order], ew[order], es[order]
        starts = np.r_[0, np.flatnonzero(np.diff(sd)) + 1]
        glen = np.diff(np.r_[starts, len(sd)])
        j = np.arange(len(sd)) - np.repeat(starts, glen)
        col = C0[sd >> 7] + j
        prow = sd & 127
        ig = np.zeros((P, ctot), dtype=np.int16)
        we = np.zeros((P, ctot), dtype=np.float32)
        wo = np.zeros((P, ctot), dtype=np.float32)
        ig[prow, col] = pair_idx[se].astype(np.int16)
        h = half[se]
        we[prow, col] = np.where(h == 0, sw_, 0.0)
        wo[prow, col] = np.where(h == 1, sw_, 0.0)
        idx_grids.append(ig)
        we_grids.append(we)
        wo_grids.append(wo)

    layout = dict(D_t=D_t, C0=C0, ctot=ctot, stages=stages, pos=pos, owner=owner)
    return layout, idx_grids, we_grids, wo_grids


def _wrap_idx(ig):
    """[128, C] chunk grid -> wrapped idx array [128, C*8] int16.

    Gather seq position i = c*128 + p; wrapped[i%16, i//16] = seq[i]."""
    seq = ig.T.reshape(-1)
    cols = seq.shape[0] // 16
    seqm = seq.reshape(cols, 16).T
    return np.tile(seqm, (8, 1)).astype(np.int16)


def _build(layout, bz1, bz2):
    import concourse.bacc as bacc
    import concourse.tile as tile
    import concourse.mybir as mybir
    from concourse.masks import make_identity

    D_t, C0 = layout["D_t"], layout["C0"]
    stages, ctot = layout["stages"], layout["ctot"]
    fp32 = mybir.dt.float32
    fp16 = mybir.dt.float16

    nc = bacc.Bacc(
        "TRN2", target_bir_lowering=False, debug=False, num_devices=NCORES,
        num_swdge_queues=4,
    )
    x_in = nc.dram_tensor("x", [F, TROWS], fp16, kind="ExternalInput")
    w1_in = nc.dram_tensor("w1", [F, HID], fp16, kind="ExternalInput")
    w2_in = nc.dram_tensor("w2", [HID, NCLS], fp32, kind="ExternalInput")
    b1_in = nc.dram_tensor("b1r", [P, HID], fp32, kind="ExternalInput")
    b2_in = nc.dram_tensor("b2r", [P, NCLS], fp32, kind="ExternalInput")
    idx_in = nc.dram_tensor("idxw", [P, ctot * 8], mybir.dt.int16, kind="ExternalInput")
    we_in = nc.dram_tensor("we", [P, ctot], fp32, kind="ExternalInput")
    wo_in = nc.dram_tensor("wo", [P, ctot], fp32, kind="ExternalInput")
    out_d = nc.dram_tensor("out", [TROWS, NCLS], fp32, kind="ExternalOutput")

    xw1_shard = nc.dram_tensor("xw1_shard", [P, PAIRS_PC], fp16)
    xw1_full = nc.dram_tensor("xw1_full", [NPAIRSG, P], fp16, addr_space="Shared")
    hw2_shard = nc.dram_tensor("hw2_shard", [P, UPAIRS * HID], fp32)
    hw2_full = nc.dram_tensor("hw2_full", [NPAIRSG, HID], fp32, addr_space="Shared")

    rg = [list(range(NCORES))]
    qrr = [0]  # round-robin SWDGE queue counter

    with tile.TileContext(nc) as tc:
        with (
            tc.tile_pool(name="const", bufs=1) as cpool,
            tc.tile_pool(name="xtp", bufs=3) as xtp,
            tc.tile_pool(name="gp", bufs=3) as gp,
            tc.tile_pool(name="gwp", bufs=2) as gwp,
            tc.tile_pool(name="hp", bufs=3) as hp,
            tc.tile_pool(name="ps", bufs=2, space="PSUM") as ps,
            tc.tile_pool(name="ps2", bufs=2, space="PSUM") as ps2,
        ):
            ident = cpool.tile([P, P], fp32)
            make_identity(nc, ident[:])
            w1t = cpool.tile([P, F // P, HID], fp16)
            nc.sync.dma_start(out=w1t[:], in_=w1_in[:].rearrange("(c p) h -> p c h", p=P))
            w2t = cpool.tile([HID, NCLS], fp32)
            nc.sync.dma_start(out=w2t[:], in_=w2_in[:])
            b1t = cpool.tile([P, HID], fp32)
            nc.sync.dma_start(out=b1t[:], in_=b1_in[:])
            b2t = cpool.tile([P, NCLS], fp32)
            nc.sync.dma_start(out=b2t[:], in_=b2_in[:])
            idxt = cpool.tile([P, ctot * 8], mybir.dt.int16)
            wet = cpool.tile([P, ctot], fp32)
            wot = cpool.tile([P, ctot], fp32)

            hs1 = cpool.tile([P, PAIRS_PC], fp16)
            nc.vector.memset(hs1[:], 0.0)
            hs2 = cpool.tile([P, UPAIRS * HID], fp32)
            nc.vector.memset(hs2[:], 0.0)
            logits = cpool.tile([P, TILES, NCLS], fp32)
            nc.vector.memset(logits[:], 0.0)

            # ---- Phase 1: XW1 = x @ W1, x loaded in 7-tile chunks ----
            XB = 7  # tiles per x chunk
            xts_list = []
            for tb in range(TILES // XB):
                xts = xtp.tile(
                    [P, F // P, XB * P], fp16, tag="xts", bufs=3,
                    name=f"xts{tb}",
                )
                nc.sync.dma_start(
                    out=xts[:],
                    in_=x_in[:, tb * XB * P : (tb + 1) * XB * P].rearrange("(c p) j -> p c j", p=P),
                )
                xts_list.append(xts)
                if tb == 0:
                    # gather-phase constants: issue after the first x chunk
                    nc.sync.dma_start(out=idxt[:], in_=idx_in[:])
                    nc.sync.dma_start(out=wet[:], in_=we_in[:])
                    nc.sync.dma_start(out=wot[:], in_=wo_in[:])
            for u in range(UPAIRS):
                mm = ps2.tile([P, 2, HID], fp32, space="PSUM", tag="mm1")
                nhalf = 2 if 2 * u + 1 < TILES else 1
                for hh in range(nhalf):
                    t = 2 * u + hh
                    tb, ti = t // XB, t % XB
                    xts = xts_list[tb]
                    for c in range(F // P):
                        nc.tensor.matmul(
                            out=mm[:, hh, :], lhsT=xts[:, c, ti * P : (ti + 1) * P], rhs=w1t[:, c, :],
                            start=(c == 0), stop=(c == F // P - 1),
                        )
                nc.any.tensor_copy(
                    hs1[:, u * P : u * P + nhalf * HID],
                    mm[:, 0:nhalf, :].rearrange("p h d -> p (h d)"),
                )
            nc.sync.dma_start(out=xw1_shard[:], in_=hs1[:])

            # ---- Phase 2: AllGather XW1 (fp16) ----
            nc.gpsimd.collective_compute(
                "AllGather", mybir.AluOpType.bypass, replica_groups=rg,
                ins=[xw1_shard[:]], outs=[xw1_full[:]],
            )

            # ---- Phases 3/5: aggregation layers ----
            GMAX = 10  # max tiles per grouped reduce

            def agg_layer(table, width, out_group_fn, elem, gdt, h1off):
                for (t0, t1, c0, c1) in stages:
                    nch = c1 - c0
                    g = gp.tile([P, STAGE_CAP, elem], gdt, tag=f"g{elem}")
                    for o in range(0, nch, CPC):
                        n = min(CPC, nch - o)
                        nc.gpsimd.dma_gather(
                            out_ap=g[:, o : o + n, :], in_ap=table[:],
                            idxs_ap=idxt[:, (c0 + o) * 8 : (c0 + o + n) * 8],
                            num_idxs=n * P, num_idxs_reg=n * P,
                            elem_size=elem, single_packet=True,
                            queue_num=qrr[0] % 4,
                        )
                        qrr[0] += 1
                    gw = gwp.tile([P, 2 * STAGE_CAP, width], gdt, tag=f"gw{elem}")
                    nc.vector.tensor_tensor(
                        out=gw[:, 0:nch, :],
                        in0=g[:, 0:nch, 0:width],
                        in1=wet[:, c0:c1].to_broadcast([P, nch, width]),
                        op=mybir.AluOpType.mult,
                    )
                    nc.vector.tensor_tensor(
                        out=gw[:, STAGE_CAP : STAGE_CAP + nch, :],
                        in0=g[:, 0:nch, h1off : h1off + width],
                        in1=wot[:, c0:c1].to_broadcast([P, nch, width]),
                        op=mybir.AluOpType.mult,
                    )
                    # contiguous half-fold: halves sum into the low block
                    nc.vector.tensor_tensor(
                        out=gw[:, 0:nch, :],
                        in0=gw[:, 0:nch, :],
                        in1=gw[:, STAGE_CAP : STAGE_CAP + nch, :],
                        op=mybir.AluOpType.add,
                    )
                    # group consecutive tiles with equal D into one reduce
                    t = t0
                    while t < t1:
                        D = int(D_t[t])
                        k = 1
                        while (
                            t + k < t1 and k < GMAX and int(D_t[t + k]) == D and D > 0
                        ):
                            k += 1
                        if D == 0:
                            t += k
                            continue
                        lo = int(C0[t]) - c0
                        red = hp.tile([P, GMAX, width], fp32, tag=f"red{width}")
                        nc.vector.tensor_reduce(
                            out=red[:, 0:k, :],
                            in_=gw[:, lo : lo + k * D, :]
                            .rearrange("p (k c) d -> p k d c", k=k),
                            axis=mybir.AxisListType.X, op=mybir.AluOpType.add,
                        )
                        out_group_fn(t, k, red)
                        t += k

            # Layer 1 epilogue: h=relu(agg+b1); hw2 = h@W2 into fp16 pair shard
            def l1_group(tg, k, red):
                if not bz1:
                    nc.vector.tensor_tensor(
                        out=red[:, 0:k, :], in0=red[:, 0:k, :],
                        in1=b1t[:, None, :].to_broadcast([P, k, HID]),
                        op=mybir.AluOpType.add,
                    )
                nc.scalar.activation(
                    red[:, 0:k, :], red[:, 0:k, :], mybir.ActivationFunctionType.Relu
                )
                for ki in range(k):
                    t = tg + ki
                    ht_ps = ps.tile([P, P], fp32, space="PSUM", tag="tp")
                    nc.tensor.transpose(out=ht_ps[0:HID, :], in_=red[:, ki, :], identity=ident[:])
                    ht = xtp.tile([HID, P], fp32, tag="ht")
                    nc.any.tensor_copy(ht[:], ht_ps[0:HID, :])
                    mm2 = ps2.tile([P, NCLS], fp32, space="PSUM", tag="mm2")
                    nc.tensor.matmul(out=mm2[:], lhsT=ht[:], rhs=w2t[:], start=True, stop=True)
                    slot = (t >> 1) * HID + (t & 1) * (HID // 2)
                    nc.any.tensor_copy(hs2[:, slot : slot + NCLS], mm2[:])

            agg_layer(xw1_full, HID, l1_group, elem=P, gdt=fp16, h1off=HID)
            nc.sync.dma_start(out=hw2_shard[:], in_=hs2[:])

            # ---- Phase 4: AllGather HW2 (fp16) ----
            nc.gpsimd.collective_compute(
                "AllGather", mybir.AluOpType.bypass, replica_groups=rg,
                ins=[hw2_shard[:]], outs=[hw2_full[:]],
            )

            # ---- Phase 5: layer 2 + batched softmax ----
            def l2_group(tg, k, red):
                if bz2:
                    nc.any.tensor_copy(logits[:, tg : tg + k, :], red[:, 0:k, :])
                else:
                    nc.vector.tensor_tensor(
                        out=logits[:, tg : tg + k, :], in0=red[:, 0:k, :],
                        in1=b2t[:, None, :].to_broadcast([P, k, NCLS]),
                        op=mybir.AluOpType.add,
                    )

            agg_layer(hw2_full, NCLS, l2_group, elem=HID, gdt=fp32, h1off=HID // 2)

            mx = cpool.tile([P, TILES], fp32)
            nc.vector.tensor_reduce(out=mx[:], in_=logits[:], axis=mybir.AxisListType.X, op=mybir.AluOpType.max)
            sh = cpool.tile([P, TILES, NCLS], fp32)
            nc.vector.tensor_tensor(
                out=sh[:], in0=logits[:],
                in1=mx[:].to_broadcast([P, TILES, NCLS]),
                op=mybir.AluOpType.subtract,
            )
            nc.scalar.activation(sh[:], sh[:], mybir.ActivationFunctionType.Exp)
            sm = cpool.tile([P, TILES], fp32)
            nc.vector.tensor_reduce(out=sm[:], in_=sh[:], axis=mybir.AxisListType.X, op=mybir.AluOpType.add)
            nc.vector.reciprocal(sm[:], sm[:])
            nc.vector.tensor_tensor(
                out=sh[:], in0=sh[:],
                in1=sm[:].to_broadcast([P, TILES, NCLS]),
                op=mybir.AluOpType.mult,
            )
            nc.sync.dma_start(
                out=out_d[:].rearrange("(t p) c -> p t c", p=P), in_=sh[:]
            )
    nc.compile()
    return nc


def kernel(x, src, dst, edge_weight, W1, b1, W2, b2):
    global LAST_EXEC_NS
    from concourse import bass_utils

    x = np.asarray(x, dtype=np.float32)
    W1 = np.asarray(W1, dtype=np.float32)
    b1 = np.asarray(b1, dtype=np.float32)
    W2 = np.asarray(W2, dtype=np.float32)
    b2 = np.asarray(b2, dtype=np.float32)

    layout, idx_grids, we_grids, wo_grids = _preprocess(src, dst, edge_weight)
    pos, owner = layout["pos"], layout["owner"]

    bz1 = bool(np.all(b1 == 0))
    bz2 = bool(np.all(b2 == 0))
    nc = _build(layout, bz1, bz2)

    b1r = np.broadcast_to(b1, (P, HID)).astype(np.float32).copy()
    b2r = np.broadcast_to(b2, (P, NCLS)).astype(np.float32).copy()
    in_maps = []
    for r in range(NCORES):
        xr = np.zeros((TROWS, F), dtype=np.float16)
        gl = np.flatnonzero(owner == r)
        xr[pos[gl]] = x[gl].astype(np.float16)
        xr = np.ascontiguousarray(xr.T)
        in_maps.append(
            {
                "x": xr,
                "w1": W1.astype(np.float16),
                "w2": W2,
                "b1r": b1r, "b2r": b2r,
                "idxw": _wrap_idx(idx_grids[r]),
                "we": we_grids[r], "wo": wo_grids[r],
            }
        )
    res = bass_utils.run_bass_kernel_spmd(
        nc, in_maps, core_ids=list(range(NCORES)), trace=_TRACE
    )
    LAST_EXEC_NS = res.exec_time_ns
    out = np.empty((N, NCLS), dtype=np.float32)
    for r in range(NCORES):
        shard = res.results[r]["out"]
        gl = np.flatnonzero(owner == r)
        out[gl] = shard[pos[gl]]
    return out


# revision 5
# speedup vs baseline: 1.0036x; 1.0036x over previous
"""2-layer GCN on 8 Trainium2 NeuronCores (Bass/Tile, SPMD).

softmax(A @ relu(A @ (X@W1) + b1) @ W2 + b2), N=50k nodes, E=800k edges.

Strategy (1D graph partition, fp16 pair-table gathers on 4 SWDGE queues):
- Nodes get a global degree rank; rank k -> core k%8, local pos k//8, so all
  cores hold near-identical degree profiles and within-core order is
  degree-descending (tight per-tile max in-degree).
- Edges partitioned by dst owner. Slot grid per core: tile t (128 dsts),
  chunk j = j-th in-edge of each dst; per-tile chunk count D_t is the
  cross-core max in-degree of that tile (degree sort keeps padding ~7%).
- Gather tables are fp16 with rows PAIRED: element = 256B = [row of tile 2u
  | row of tile 2u+1] at partition q; pair index o*3200 + q*25 + u < 25600
  fits int16 (no low/high split). The weighted select between the two
  halves uses two per-slot weight grids (we: half0, wo: half1).
- dma_gather calls (<=8 chunks = 1024 idxs) round-robin over 4 SWDGE
  queues; Q7 descriptor generation for different queues runs on different
  GpSimd core pairs concurrently (~3x issue throughput vs 1 queue).
- Layer-2 table HW2 is 16 classes padded to the same 64-col pair layout, so
  both layers share identical idx/weight grids.
- XW1/HW2 shards are fp16 [128, 3200] built in SBUF, one DMA out, AllGather
  (half the bytes of fp32), gathered as [25600, 128] pair-rows.
"""

import os
import sys

sys.path.insert(0, "/opt/trn_rl_repo")

import numpy as np

N = 50000
E = 800000
F = 512
HID = 64
NCLS = 16
NCORES = 8
P = 128
NPC = N // NCORES  # 6250
TILES = 49
TROWS = TILES * P  # 6272
UPAIRS = 25  # ceil(TILES/2) pair-ranks per core
PAIRS_PC = UPAIRS * P  # 3200
NPAIRSG = NCORES * PAIRS_PC  # 25600
STAGE_CAP = 48  # chunks per stage (6 gather calls)
CPC = 8  # chunks per gather call (1024 idxs)

_TRACE = False
LAST_EXEC_NS = None


def _preprocess(src, dst, edge_weight):
    src = np.asarray(src).astype(np.int64).ravel()
    dst = np.asarray(dst).astype(np.int64).ravel()
    w = np.asarray(edge_weight).astype(np.float32).ravel()

    tdeg = np.bincount(dst, minlength=N)
    grank = np.empty(N, dtype=np.int64)
    grank[np.argsort(-tdeg, kind="stable")] = np.arange(N)
    owner = grank % NCORES
    pos = grank // NCORES  # degree-descending within core

    # node -> gather pair index / half
    tl = pos >> 7
    q = pos & 127
    u = tl >> 1
    pair_idx = owner * PAIRS_PC + q * UPAIRS + u  # < 25600
    half = tl & 1

    owner_dst = owner[dst]

    # per-tile chunk counts: cross-core max of per-node in-degree maxima
    D_t = np.zeros(TILES, dtype=np.int64)
    per_core = []
    for r in range(NCORES):
        m = owner_dst == r
        es, ew = src[m], w[m]
        dl = pos[dst[m]]
        cnt = np.bincount(dl, minlength=NPC)
        cnt_pad = np.concatenate([cnt, np.zeros(TROWS - NPC, np.int64)])
        D_t = np.maximum(D_t, cnt_pad.reshape(TILES, P).max(1))
        per_core.append((es, ew, dl))

    C0 = np.zeros(TILES, dtype=np.int64)  # tile -> first chunk column
    C0[1:] = np.cumsum(D_t)[:-1]
    ctot = int(D_t.sum())

    # stages: consecutive tiles, <= STAGE_CAP chunks each
    stages = []  # (t0, t1, c0, c1)
    t0 = 0
    while t0 < TILES:
        t1 = t0
        nch = 0
        while t1 < TILES and nch + D_t[t1] <= STAGE_CAP:
            nch += D_t[t1]
            t1 += 1
        if t1 == t0:  # single tile bigger than cap (shouldn't happen)
            t1 = t0 + 1
            nch = D_t[t0]
        stages.append((t0, t1, int(C0[t0]), int(C0[t0] + nch)))
        t0 = t1

    idx_grids, we_grids, wo_grids = [], [], []
    for r in range(NCORES):
        es, ew, dl = per_core[r]
        order = np.argsort(dl, kind="stable")
        sd, sw_, se = dl[order], ew[order], es[order]
        starts = np.r_[0, np.flatnonzero(np.diff(sd)) + 1]
        glen = np.diff(np.r_[starts, len(sd)])
        j = np.arange(len(sd)) - np.repeat(starts, glen)
        col = C0[sd >> 7] + j
        prow = sd & 127
        ig = np.zeros((P, ctot), dtype=np.int16)
        we = np.zeros((P, ctot), dtype=np.float32)
        wo = np.zeros((P, ctot), dtype=np.float32)
        ig[prow, col] = pair_idx[se].astype(np.int16)
        h = half[se]
        we[prow, col] = np.where(h == 0, sw_, 0.0)
        wo[prow, col] = np.where(h == 1, sw_, 0.0)
        idx_grids.append(ig)
        we_grids.append(we)
        wo_grids.append(wo)

    layout = dict(D_t=D_t, C0=C0, ctot=ctot, stages=stages, pos=pos, owner=owner)
    return layout, idx_grids, we_grids, wo_grids


def _wrap_idx(ig):
    """[128, C] chunk grid -> wrapped idx array [128, C*8] int16.

    Gather seq position i = c*128 + p; wrapped[i%16, i//16] = seq[i]."""
    seq = ig.T.reshape(-1)
    cols = seq.shape[0] // 16
    seqm = seq.reshape(cols, 16).T
    return np.tile(seqm, (8, 1)).astype(np.int16)


def _build(layout, bz1, bz2):
    import concourse.bacc as bacc
    import concourse.tile as tile
    import concourse.mybir as mybir
    from concourse.masks import make_identity

    D_t, C0 = layout["D_t"], layout["C0"]
    stages, ctot = layout["stages"], layout["ctot"]
    fp32 = mybir.dt.float32
    fp16 = mybir.dt.float16

    nc = bacc.Bacc(
        "TRN2", target_bir_lowering=False, debug=False, num_devices=NCORES,
        num_swdge_queues=4,
    )
    x_in = nc.dram_tensor("x", [F, TROWS], fp16, kind="ExternalInput")
    w1_in = nc.dram_tensor("w1", [F, HID], fp16, kind="ExternalInput")
    w2_in = nc.dram_tensor("w2", [HID, NCLS], fp32, kind="ExternalInput")
    b1_in = nc.dram_tensor("b1r", [P, HID], fp32, kind="ExternalInput")
    b2_in = nc.dram_tensor("b2r", [P, NCLS], fp32, kind="ExternalInput")
    idx_in = nc.dram_tensor("idxw", [P, ctot * 8], mybir.dt.int16, kind="ExternalInput")
    we_in = nc.dram_tensor("we", [P, ctot], fp32, kind="ExternalInput")
    wo_in = nc.dram_tensor("wo", [P, ctot], fp32, kind="ExternalInput")
    out_d = nc.dram_tensor("out", [TROWS, NCLS], fp32, kind="ExternalOutput")

    xw1_shard = nc.dram_tensor("xw1_shard", [P, PAIRS_PC], fp16)
    xw1_full = nc.dram_tensor("xw1_full", [NPAIRSG, P], fp16, addr_space="Shared")
    hw2_shard = nc.dram_tensor("hw2_shard", [P, UPAIRS * HID], fp32)
    hw2_full = nc.dram_tensor("hw2_full", [NPAIRSG, HID], fp32, addr_space="Shared")

    rg = [list(range(NCORES))]
    qrr = [0]  # round-robin SWDGE queue counter

    with tile.TileContext(nc) as tc:
        with (
            tc.tile_pool(name="const", bufs=1) as cpool,
            tc.tile_pool(name="xtp", bufs=3) as xtp,
            tc.tile_pool(name="gp", bufs=3) as gp,
            tc.tile_pool(name="gwp", bufs=2) as gwp,
            tc.tile_pool(name="hp", bufs=3) as hp,
            tc.tile_pool(name="ps", bufs=2, space="PSUM") as ps,
            tc.tile_pool(name="ps2", bufs=2, space="PSUM") as ps2,
        ):
            ident = cpool.tile([P, P], fp32)
            make_identity(nc, ident[:])
            w1t = cpool.tile([P, F // P, HID], fp16)
            nc.sync.dma_start(out=w1t[:], in_=w1_in[:].rearrange("(c p) h -> p c h", p=P))
            w2t = cpool.tile([HID, NCLS], fp32)
            nc.sync.dma_start(out=w2t[:], in_=w2_in[:])
            b1t = cpool.tile([P, HID], fp32)
            nc.sync.dma_start(out=b1t[:], in_=b1_in[:])
            b2t = cpool.tile([P, NCLS], fp32)
            nc.sync.dma_start(out=b2t[:], in_=b2_in[:])
            idxt = cpool.tile([P, ctot * 8], mybir.dt.int16)
            wet = cpool.tile([P, ctot], fp32)
            wot = cpool.tile([P, ctot], fp32)

            hs1 = cpool.tile([P, PAIRS_PC], fp16)
            nc.vector.memset(hs1[:], 0.0)
            hs2 = cpool.tile([P, UPAIRS * HID], fp32)
            nc.vector.memset(hs2[:], 0.0)
            logits = cpool.tile([P, TILES, NCLS], fp32)
            nc.vector.memset(logits[:], 0.0)

            # ---- Phase 1: XW1 = x @ W1, x loaded in 7-tile chunks ----
            XB = 7  # tiles per x chunk
            xts_list = []
            for tb in range(TILES // XB):
                xts = xtp.tile(
                    [P, F // P, XB * P], fp16, tag="xts", bufs=3,
                    name=f"xts{tb}",
                )
                nc.sync.dma_start(
                    out=xts[:],
                    in_=x_in[:, tb * XB * P : (tb + 1) * XB * P].rearrange("(c p) j -> p c j", p=P),
                )
                xts_list.append(xts)
                if tb == 0:
                    # gather-phase constants on the idle SWDGE ring so the
                    # hs1 shard DMA doesn't queue behind them on sync
                    nc.gpsimd.dma_start(out=idxt[:], in_=idx_in[:])
                    nc.gpsimd.dma_start(out=wet[:], in_=we_in[:])
                    nc.gpsimd.dma_start(out=wot[:], in_=wo_in[:])
            for u in range(UPAIRS):
                mm = ps2.tile([P, 2, HID], fp32, space="PSUM", tag="mm1")
                nhalf = 2 if 2 * u + 1 < TILES else 1
                for hh in range(nhalf):
                    t = 2 * u + hh
                    tb, ti = t // XB, t % XB
                    xts = xts_list[tb]
                    for c in range(F // P):
                        nc.tensor.matmul(
                            out=mm[:, hh, :], lhsT=xts[:, c, ti * P : (ti + 1) * P], rhs=w1t[:, c, :],
                            start=(c == 0), stop=(c == F // P - 1),
                        )
                nc.any.tensor_copy(
                    hs1[:, u * P : u * P + nhalf * HID],
                    mm[:, 0:nhalf, :].rearrange("p h d -> p (h d)"),
                )
            nc.sync.dma_start(out=xw1_shard[:], in_=hs1[:])

            # ---- Phase 2: AllGather XW1 (fp16) ----
            nc.gpsimd.collective_compute(
                "AllGather", mybir.AluOpType.bypass, replica_groups=rg,
                ins=[xw1_shard[:]], outs=[xw1_full[:]],
            )

            # ---- Phases 3/5: aggregation layers ----
            GMAX = 10  # max tiles per grouped reduce

            def agg_layer(table, width, out_group_fn, elem, gdt, h1off):
                for (t0, t1, c0, c1) in stages:
                    nch = c1 - c0
                    g = gp.tile([P, STAGE_CAP, elem], gdt, tag=f"g{elem}")
                    for o in range(0, nch, CPC):
                        n = min(CPC, nch - o)
                        nc.gpsimd.dma_gather(
                            out_ap=g[:, o : o + n, :], in_ap=table[:],
                            idxs_ap=idxt[:, (c0 + o) * 8 : (c0 + o + n) * 8],
                            num_idxs=n * P, num_idxs_reg=n * P,
                            elem_size=elem, single_packet=True,
                            queue_num=qrr[0] % 4,
                        )
                        qrr[0] += 1
                    gw = gwp.tile([P, 2 * STAGE_CAP, width], gdt, tag=f"gw{elem}")
                    nc.vector.tensor_tensor(
                        out=gw[:, 0:nch, :],
                        in0=g[:, 0:nch, 0:width],
                        in1=wet[:, c0:c1].to_broadcast([P, nch, width]),
                        op=mybir.AluOpType.mult,
                    )
                    nc.vector.tensor_tensor(
                        out=gw[:, STAGE_CAP : STAGE_CAP + nch, :],
                        in0=g[:, 0:nch, h1off : h1off + width],
                        in1=wot[:, c0:c1].to_broadcast([P, nch, width]),
                        op=mybir.AluOpType.mult,
                    )
                    # contiguous half-fold: halves sum into the low block
                    nc.vector.tensor_tensor(
                        out=gw[:, 0:nch, :],
                        in0=gw[:, 0:nch, :],
                        in1=gw[:, STAGE_CAP : STAGE_CAP + nch, :],
                        op=mybir.AluOpType.add,
                    )
                    # group consecutive tiles with equal D into one reduce
                    t = t0
                    while t < t1:
                        D = int(D_t[t])
                        k = 1
                        while (
                            t + k < t1 and k < GMAX and int(D_t[t + k]) == D and D > 0
                        ):
                            k += 1
                        if D == 0:
                            t += k
                            continue
                        lo = int(C0[t]) - c0
                        red = hp.tile([P, GMAX, width], fp32, tag=f"red{width}")
                        nc.vector.tensor_reduce(
                            out=red[:, 0:k, :],
                            in_=gw[:, lo : lo + k * D, :]
                            .rearrange("p (k c) d -> p k d c", k=k),
                            axis=mybir.AxisListType.X, op=mybir.AluOpType.add,
                        )
                        out_group_fn(t, k, red)
                        t += k

            # Layer 1 epilogue: h=relu(agg+b1); hw2 = h@W2 into fp16 pair shard
            def l1_group(tg, k, red):
                if not bz1:
                    nc.vector.tensor_tensor(
                        out=red[:, 0:k, :], in0=red[:, 0:k, :],
                        in1=b1t[:, None, :].to_broadcast([P, k, HID]),
                        op=mybir.AluOpType.add,
                    )
                nc.scalar.activation(
                    red[:, 0:k, :], red[:, 0:k, :], mybir.ActivationFunctionType.Relu
                )
                for ki in range(k):
                    t = tg + ki
                    ht_ps = ps.tile([P, P], fp32, space="PSUM", tag="tp")
                    nc.tensor.transpose(out=ht_ps[0:HID, :], in_=red[:, ki, :], identity=ident[:])
                    ht = xtp.tile([HID, P], fp32, tag="ht")
                    nc.any.tensor_copy(ht[:], ht_ps[0:HID, :])
                    mm2 = ps2.tile([P, NCLS], fp32, space="PSUM", tag="mm2")
                    nc.tensor.matmul(out=mm2[:], lhsT=ht[:], rhs=w2t[:], start=True, stop=True)
                    slot = (t >> 1) * HID + (t & 1) * (HID // 2)
                    nc.any.tensor_copy(hs2[:, slot : slot + NCLS], mm2[:])

            agg_layer(xw1_full, HID, l1_group, elem=P, gdt=fp16, h1off=HID)
            nc.sync.dma_start(out=hw2_shard[:], in_=hs2[:])

            # ---- Phase 4: AllGather HW2 (fp16) ----
            nc.gpsimd.collective_compute(
                "AllGather", mybir.AluOpType.bypass, replica_groups=rg,
                ins=[hw2_shard[:]], outs=[hw2_full[:]],
            )

            # ---- Phase 5: layer 2 + batched softmax ----
            def l2_group(tg, k, red):
                if bz2:
                    nc.any.tensor_copy(logits[:, tg : tg + k, :], red[:, 0:k, :])
                else:
                    nc.vector.tensor_tensor(
                        out=logits[:, tg : tg + k, :], in0=red[:, 0:k, :],
                        in1=b2t[:, None, :].to_broadcast([P, k, NCLS]),
                        op=mybir.AluOpType.add,
                    )

            agg_layer(hw2_full, NCLS, l2_group, elem=HID, gdt=fp32, h1off=HID // 2)

            mx = cpool.tile([P, TILES], fp32)
            nc.vector.tensor_reduce(out=mx[:], in_=logits[:], axis=mybir.AxisListType.X, op=mybir.AluOpType.max)
            sh = cpool.tile([P, TILES, NCLS], fp32)
            nc.vector.tensor_tensor(
                out=sh[:], in0=logits[:],
                in1=mx[:].to_broadcast([P, TILES, NCLS]),
                op=mybir.AluOpType.subtract,
            )
            nc.scalar.activation(sh[:], sh[:], mybir.ActivationFunctionType.Exp)
            sm = cpool.tile([P, TILES], fp32)
            nc.vector.tensor_reduce(out=sm[:], in_=sh[:], axis=mybir.AxisListType.X, op=mybir.AluOpType.add)
            nc.vector.reciprocal(sm[:], sm[:])
            nc.vector.tensor_tensor(
                out=sh[:], in0=sh[:],
                in1=sm[:].to_broadcast([P, TILES, NCLS]),
                op=mybir.AluOpType.mult,
            )
            nc.sync.dma_start(
                out=out_d[:].rearrange("(t p) c -> p t c", p=P), in_=sh[:]
            )
    nc.compile()
    return nc


def kernel(x, src, dst, edge_weight, W1, b1, W2, b2):
    global LAST_EXEC_NS
    from concourse import bass_utils

    x = np.asarray(x, dtype=np.float32)
    W1 = np.asarray(W1, dtype=np.float32)
    b1 = np.asarray(b1, dtype=np.float32)
    W2 = np.asarray(W2, dtype=np.float32)
    b2 = np.asarray(b2, dtype=np.float32)

    layout, idx_grids, we_grids, wo_grids = _preprocess(src, dst, edge_weight)
    pos, owner = layout["pos"], layout["owner"]

    bz1 = bool(np.all(b1 == 0))
    bz2 = bool(np.all(b2 == 0))
    nc = _build(layout, bz1, bz2)

    b1r = np.broadcast_to(b1, (P, HID)).astype(np.float32).copy()
    b2r = np.broadcast_to(b2, (P, NCLS)).astype(np.float32).copy()
    in_maps = []
    for r in range(NCORES):
        xr = np.zeros((TROWS, F), dtype=np.float16)
        gl = np.flatnonzero(owner == r)
        xr[pos[gl]] = x[gl].astype(np.float16)
        xr = np.ascontiguousarray(xr.T)
        in_maps.append(
            {
                "x": xr,
                "w1": W1.astype(np.float16),
                "w2": W2,
                "b1r": b1r, "b2r": b2r,
                "idxw": _wrap_idx(idx_grids[r]),
                "we": we_grids[r], "wo": wo_grids[r],
            }
        )
    res = bass_utils.run_bass_kernel_spmd(
        nc, in_maps, core_ids=list(range(NCORES)), trace=_TRACE
    )
    LAST_EXEC_NS = res.exec_time_ns
    out = np.empty((N, NCLS), dtype=np.float32)
    for r in range(NCORES):
        shard = res.results[r]["out"]
        gl = np.flatnonzero(owner == r)
        out[gl] = shard[pos[gl]]
    return out
